# revision 1
# baseline (speedup 1.0000x reference)
"""Trainium2 Bass kernel for nn_AttentionFusion (dense_transformer).

Pure data parallel across 8 NeuronCores: batch 16384 is split into 8 shards
of 2048 rows; weights (~7MB) are replicated on every core.  Each core runs
an identical fused program:

  proj(v,t,a) -> 3-token seq -> MHA (seq_len=3, 4 heads) -> +res -> LN ->
  FFN(relu) -> +res -> LN -> mean-pool over tokens -> + orig @ Wo.T + bo

Per-core design (512-row supertiles):
  - All matmuls on PE in bf16 with fp32 PSUM accumulation.  Activations are
    kept in natural layout [128 rows, feat] for VectorE/ScalarE work;
    feature-on-partition ("transposed") bf16 copies for matmul lhsT are
    produced by the DMA xbar transpose engine, one batched instruction per
    [128 x 512] (or wider) natural block using layout [P, j, k, 128].
  - FFN1 emits a transposed hidden so the per-feature bias+relu fuse into
    ScalarE's PSUM->SBUF copyback (bias is per-partition there).
  - Residual adds ride the PE via identity-matmul accumulation into PSUM;
    LayerNorm stats use VectorE bn_stats/bn_aggr straight off PSUM;
    rsqrt = exp(-0.5*ln(var+eps)) on ScalarE.
  - The activation-table registry is monkeypatched down to the single set
    `natural_log_exp_and_others` (contains ln/exp/identity/copy/relu) so
    ScalarE never reloads tables (~2.7us each) mid-kernel.
  - 1/sqrt(HD) is folded into the q projection weights/bias at init.
  - Constant bias vectors enter PSUM via K=1 matmuls against a ones column.
"""

import os
import sys

for _p in ("/opt/trn_rl_repo",):
    if _p not in sys.path and os.path.isdir(_p):
        sys.path.insert(0, _p)

import numpy as np

import concourse.bacc as bacc
import concourse.mybir as mybir
import concourse.tile as tile
from concourse.bass_utils import run_bass_kernel_spmd
from concourse.masks import make_identity

# Pin ScalarE to one activation-table set: keep only natural_log_exp_and_others
# populated (its index must stay aligned with act_info.json, so other sets are
# emptied rather than removed).
import concourse.hw_specs as _hw_specs

_ORIG_GET_TABLES = _hw_specs.get_activation_tables
_KEEP_SET = "natural_log_exp_and_others"


def _pinned_tables(module_arch):
    t = _ORIG_GET_TABLES(module_arch)
    if _KEEP_SET in t:
        t = {k: (v if k == _KEEP_SET else set()) for k, v in t.items()}
    return t


bacc.get_activation_tables = _pinned_tables

# Problem constants (hardcoded per harness contract).
B, H, NH, HD = 16384, 512, 4, 128
FF = 4 * H
EPS = 1e-5
N_CORES = 8
B_CORE = B // N_CORES  # 2048
P = 128
ST = 512               # supertile rows
NB = ST // P           # batch sub-tiles per supertile
KH = H // P            # k-chunks over hidden dim
KF = FF // P           # k-chunks over FF dim

FP32 = mybir.dt.float32
BF16 = mybir.dt.bfloat16
AX = mybir.AxisListType
OP = mybir.AluOpType
AF = mybir.ActivationFunctionType


def build_nc(b_core=B_CORE, ln1_affine=False, ln2_affine=False, passes=1):
    """Build the per-core Bass program."""
    nst = b_core // ST
    assert nst * ST == b_core

    nc = bacc.Bacc("TRN2", target_bir_lowering=False, debug=False, num_devices=1)

    # ---- DRAM I/O ----
    vf = nc.dram_tensor("visual_feat", (b_core, H), FP32, kind="ExternalInput")
    tf = nc.dram_tensor("tactile_feat", (b_core, H), FP32, kind="ExternalInput")
    af = nc.dram_tensor("arm_feat", (b_core, H), FP32, kind="ExternalInput")
    wd = {
        "Wv": nc.dram_tensor("Wv", (H, H), FP32, kind="ExternalInput"),
        "Wt": nc.dram_tensor("Wt", (H, H), FP32, kind="ExternalInput"),
        "Wa": nc.dram_tensor("Wa", (H, H), FP32, kind="ExternalInput"),
        "in_proj_w": nc.dram_tensor("in_proj_w", (3 * H, H), FP32, kind="ExternalInput"),
        "out_w": nc.dram_tensor("out_w", (H, H), FP32, kind="ExternalInput"),
        "w1": nc.dram_tensor("w1", (FF, H), FP32, kind="ExternalInput"),
        "w2": nc.dram_tensor("w2", (H, FF), FP32, kind="ExternalInput"),
        "Wo": nc.dram_tensor("Wo", (H, 3 * H), FP32, kind="ExternalInput"),
    }
    bd = {}
    for nm, sz in [("bv", H), ("bt", H), ("ba", H), ("in_proj_b", 3 * H),
                   ("out_b", H), ("b1", FF), ("b2", H), ("g1", H), ("be1", H),
                   ("g2", H), ("be2", H), ("bo", H)]:
        bd[nm] = nc.dram_tensor(nm, (sz,), FP32, kind="ExternalInput")
    out_d = nc.dram_tensor("out", (b_core, H), FP32, kind="ExternalOutput")

    feats = [vf, tf, af]
    inv_sqrt_hd = float(1.0 / np.sqrt(HD))

    with tile.TileContext(nc) as tc:
        with tc.tile_pool(name="const", bufs=1) as cpool, \
             tc.tile_pool(name="ps", bufs=8, space="PSUM") as pspool:
            # ================= init (wstage pool is recycled) =============
            with tc.tile_pool(name="wstage", bufs=1) as wpool:
                # ---- tiny constants & bias tiles (DMAs head the SP queue;
                # broadcasts ride a PE ones-matmul so GPSIMD stays free for
                # the weight casts) ----
                ones_col = cpool.tile([1, P], BF16, tag="ones", name="ones_col")
                nc.vector.memset(ones_col[:], 1.0)
                eps_pp = cpool.tile([P, 1], FP32, tag="eps", name="eps_pp")
                nc.vector.memset(eps_pp[:], EPS)
                ident = cpool.tile([P, P], BF16, tag="ident", name="ident")
                make_identity(nc, ident[:])

                def bcast_tile(name, src_ap, n, dt=BF16):
                    bt_ = cpool.tile([P, n], dt, tag=f"bc_{name}", name=f"{name}_bc")
                    for s0 in range(0, n, H):
                        row = wpool.tile([1, H], FP32, tag="brow", bufs=2,
                                         name=f"{name}_row{s0}")
                        nc.sync.dma_start(row[:], src_ap[None, s0:s0 + H])
                        rowc = wpool.tile([1, H], BF16, tag="browc", bufs=2,
                                          name=f"{name}_rowc{s0}")
                        nc.vector.tensor_copy(rowc[:], row[:])
                        bps = pspool.tile([P, H], FP32, tag="ps",
                                          name=f"{name}_ps{s0}")
                        nc.tensor.matmul(bps[:], ones_col[:], rowc[:],
                                         start=True, stop=True)
                        nc.scalar.copy(bt_[:, s0:s0 + H], bps[:])
                    return bt_

                bmod = [bcast_tile(nm, bd[nm], H) for nm in ("bv", "bt", "ba")]

                def brow_bf(name, src_ap, n, scale=None):
                    rowf = wpool.tile([1, n], FP32, tag="brow", bufs=2,
                                      name=f"{name}_rowf")
                    nc.sync.dma_start(rowf[:], src_ap)
                    rowb = cpool.tile([1, n], BF16, tag=f"br_{name}",
                                      name=f"{name}_rowb")
                    if scale is None:
                        nc.vector.tensor_copy(rowb[:], rowf[:])
                    else:
                        nc.vector.tensor_scalar_mul(rowb[:], rowf[:], scale)
                    return rowb

                ipbq_row = brow_bf("ipbq", bd["in_proj_b"][None, 0:H], H,
                                   scale=inv_sqrt_hd)
                ipbk_row = brow_bf("ipbk", bd["in_proj_b"][None, H:2 * H], H)
                ipbv_row = brow_bf("ipbv", bd["in_proj_b"][None, 2 * H:], H)
                ipb_rows = [ipbq_row, ipbk_row, ipbv_row]
                outb_row = brow_bf("outb", bd["out_b"][None, :], H)
                b2_row = brow_bf("b2", bd["b2"][None, :], H)

                # bo_eff = bo + be2 (be2 enters pooled exactly once post-mean)
                bo_rowf = wpool.tile([1, H], FP32, tag="brow", bufs=2,
                                     name="bo_rowf")
                nc.sync.dma_start(bo_rowf[:], bd["bo"][None, :])
                be2_rowf = wpool.tile([1, H], FP32, tag="brow", bufs=2,
                                      name="be2_rowf")
                nc.sync.dma_start(be2_rowf[:], bd["be2"][None, :])
                boe_row = cpool.tile([1, H], BF16, tag="br_boe", name="boe_rowb")
                nc.vector.tensor_add(boe_row[:], bo_rowf[:], be2_rowf[:])

                # b1 in per-partition layout [128, KF]
                b1_pp = cpool.tile([P, KF], FP32, tag="b1pp", name="b1_pp")
                nc.sync.dma_start(b1_pp[:], bd["b1"].rearrange("(c p) -> p c", p=P))

                g1_bc = be1_bc = g2_bc = None
                if ln1_affine:
                    g1_bc = bcast_tile("g1", bd["g1"], H)
                    be1_bc = bcast_tile("be1", bd["be1"], H)
                if ln2_affine:
                    g2_bc = bcast_tile("g2", bd["g2"], H)

                # ---- weights: PE-transpose (fp32) + ScalarE bf16 copyback.
                # The PE is idle during init and this keeps the DMA stream
                # copy-only (no xbar transpose<->copy mode serialization). ----
                ident_f = wpool.tile([P, P], FP32, tag="identf", name="ident_f")
                make_identity(nc, ident_f[:])

                def prep_weight(name, dram, n_out, n_in):
                    """W.T in [P(=in chunk), kc_n, n_out] bf16."""
                    oc_n = n_out // P
                    kc_n = n_in // P
                    nat = wpool.tile([P, oc_n, n_in], FP32, tag="wstage", bufs=2,
                                     name=f"{name}_nat")
                    nc.sync.dma_start(nat[:], dram.rearrange("(c p) f -> p c f", p=P))
                    wt = cpool.tile([P, kc_n, n_out], BF16, tag=f"wt_{name}",
                                    name=f"{name}_T")
                    for k in range(kc_n):
                        for og in range(0, n_out, 512):
                            gw = min(512, n_out - og)
                            tp = pspool.tile([P, 512], FP32, tag="ps",
                                             name=f"tp_{name}_{k}_{og}")
                            for oc in range(og // P, (og + gw) // P):
                                nc.tensor.transpose(
                                    tp[:, (oc * P - og):(oc * P - og) + P],
                                    nat[:, oc, k * P:(k + 1) * P],
                                    ident_f[:])
                            nc.scalar.copy(wt[:, k, og:og + gw], tp[:, :gw])
                    return wt

                # prep in PE-consumption order: proj, early-final, qkv,
                # out_proj, ffn1, ffn2
                wvT = prep_weight("Wv", wd["Wv"], H, H)
                wtT = prep_weight("Wt", wd["Wt"], H, H)
                waT = prep_weight("Wa", wd["Wa"], H, H)
                woT = prep_weight("wo", wd["Wo"], H, 3 * H)
                ipwT = prep_weight("ipw", wd["in_proj_w"], 3 * H, H)
                owT = prep_weight("ow", wd["out_w"], H, H)
                w1T = prep_weight("w1", wd["w1"], FF, H)
                w2T = prep_weight("w2", wd["w2"], H, FF)
                wTs = [wvT, wtT, waT]

                # q weights absorb the 1/sqrt(HD) score scale (GPSIMD: keeps
                # VectorE's stream free of weight-load dependencies)
                nc.gpsimd.tensor_scalar_mul(ipwT[:, :, 0:H], ipwT[:, :, 0:H],
                                            inv_sqrt_hd)

            main_pools = (
                tc.tile_pool(name="act", bufs=1),
                tc.tile_pool(name="rot", bufs=3),
            )
            apool = main_pools[0].__enter__()
            rpool = main_pools[1].__enter__()

            qkv_bufs = 1 if (ln1_affine or ln2_affine) else 2

            # ---- helpers ----
            def layer_norm(ps, dst, tagp, affine, g_bc, be_bc):
                """dst = LN(ps)[*g + be]; ps is PSUM fp32 holding x+residual."""
                bns = rpool.tile([P, 6], FP32, tag="lns6", bufs=4, name=f"b_{tagp}")
                nc.vector.bn_stats(bns[:], ps[:])
                bna = rpool.tile([P, 2], FP32, tag="lns2", bufs=3, name=f"a_{tagp}")
                nc.vector.bn_aggr(bna[:], bns[:])
                mean, var = bna[:, 0:1], bna[:, 1:2]
                lnv = rpool.tile([P, 1], FP32, tag="lns", bufs=6, name=f"lv_{tagp}")
                nc.scalar.activation(lnv[:], var, AF.Ln, bias=eps_pp[:])
                rstd = rpool.tile([P, 1], FP32, tag="lns", bufs=6, name=f"rs_{tagp}")
                nc.scalar.activation(rstd[:], lnv[:], AF.Exp, scale=-0.5)
                nmr = rpool.tile([P, 1], FP32, tag="lns", bufs=6, name=f"nm_{tagp}")
                nc.vector.tensor_scalar(
                    nmr[:], mean, rstd[:], -1.0, op0=OP.mult, op1=OP.mult)
                if not affine:
                    nc.scalar.activation(dst, ps[:], AF.Identity,
                                         bias=nmr[:], scale=rstd[:])
                else:
                    nrm = rpool.tile([P, H], BF16, tag="lnnrm", bufs=2,
                                     name=f"nr_{tagp}")
                    nc.scalar.activation(nrm[:], ps[:], AF.Identity,
                                         bias=nmr[:], scale=rstd[:])
                    nc.vector.tensor_mul(dst, nrm[:], g_bc[:])
                    if be_bc is not None:
                        nc.vector.tensor_add(dst, dst, be_bc[:])

            # ================= main loop =================
            # Transposed activations use layout [P, NB(j), KH(k), 128]:
            # ZT[p, j, k, b] = Z[j*128+b, k*128+p]; a whole [128, NB*H]
            # natural block transposes in ONE xbar instruction.
            for st_ in range(nst * passes):
                st = st_ % nst
                r0 = st * ST

                # ---- stage 0: feats load + cast + batched transpose ----
                featsT = []
                for m in range(3):
                    fT = apool.tile([P, NB, KH, P], BF16, tag=f"fT{m}",
                                    name=f"fT{st_}_{m}")
                    for j in range(NB):
                        fnat = rpool.tile([P, H], FP32, tag="fnat", bufs=2,
                                          name=f"fn{st_}_{m}_{j}")
                        nc.scalar.dma_start(
                            fnat[:], feats[m][r0 + j * P:r0 + (j + 1) * P, :])
                        fbf = rpool.tile([P, H], BF16, tag="fbf", bufs=2,
                                         name=f"fb{st_}_{m}_{j}")
                        nc.scalar.copy(fbf[:], fnat[:])
                        nc.scalar.dma_start_transpose(fT[:, j], fbf[:])
                    featsT.append(fT)

                # ---- stage 1: modality projections -> combined (natural) ----
                comb_nat = apool.tile([P, 3, NB, H], BF16, tag="combn",
                                      name=f"combn{st_}")
                for m in range(3):
                    for j in range(NB):
                        ps = pspool.tile([P, H], FP32, tag="ps",
                                         name=f"ps_pj{st_}_{m}_{j}")
                        for k in range(KH):
                            nc.tensor.matmul(
                                ps[:], featsT[m][:, j, k, :],
                                wTs[m][:, k, :], start=(k == 0), stop=(k == KH - 1))
                        nc.vector.tensor_add(comb_nat[:, m, j, :], ps[:], bmod[m][:])

                # early final projection: orig @ Wo.T + bo_eff -> outt
                # (frees featsT so the next supertile's feat prep can start)
                outt = apool.tile([P, NB, H], FP32, tag="outt", bufs=1,
                                  name=f"ot{st_}")
                for j in range(NB):
                    ps = pspool.tile([P, H], FP32, tag="ps", name=f"ps_fi{st_}_{j}")
                    nc.tensor.matmul(ps[:], ones_col[:], boe_row[:],
                                     start=True, stop=False)
                    for m in range(3):
                        for k in range(KH):
                            nc.tensor.matmul(
                                ps[:], featsT[m][:, j, k, :],
                                woT[:, m * KH + k, :], start=False,
                                stop=(m == 2 and k == KH - 1))
                    nc.scalar.copy(outt[:, j, :], ps[:])

                combT = []
                for t in range(3):
                    cT = apool.tile([P, NB, KH, P], BF16, tag="combT", bufs=3,
                                    name=f"combT{st_}_{t}")
                    nc.scalar.dma_start_transpose(
                        cT[:].rearrange("p j k b -> p (j k) b"), comb_nat[:, t])
                    combT.append(cT)

                # ---- stages 2+3: per batch sub-tile: qkv + attention ----
                ctxT = [apool.tile([P, NB, KH, P], BF16, tag="ctxT", bufs=3,
                                   name=f"ctxT{st_}_{t}") for t in range(3)]
                for j in range(NB):
                    qkv = apool.tile([P, 3, 3, H], BF16, tag="qkv", bufs=qkv_bufs,
                                     name=f"qkv{st_}_{j}")
                    for t in range(3):
                        pss = [pspool.tile([P, H], FP32, tag="ps",
                                           name=f"ps_qk{st_}_{t}_{j}_{s3}")
                               for s3 in range(3)]
                        for s3 in range(3):
                            nc.tensor.matmul(pss[s3][:], ones_col[:],
                                             ipb_rows[s3][:],
                                             start=True, stop=False)
                        for k in range(KH):
                            for s3 in range(3):
                                nc.tensor.matmul(
                                    pss[s3][:], combT[t][:, j, k, :],
                                    ipwT[:, k, s3 * H:(s3 + 1) * H],
                                    start=False, stop=(k == KH - 1))
                        for s3 in range(3):
                            nc.scalar.copy(qkv[:, t, s3], pss[s3][:])

                    # scores[b, qt, kt, h] = sum_d q*k (q pre-scaled)
                    scores = rpool.tile([P, 3, 3, NH], FP32, tag="scores", bufs=2,
                                        name=f"sc{st_}_{j}")
                    for qt in range(3):
                        prod = rpool.tile([P, 3, H], BF16, tag="prod", bufs=1,
                                          name=f"pr{st_}_{j}_{qt}")
                        nc.vector.tensor_mul(
                            prod[:],
                            qkv[:, qt, 0, :].rearrange("p (x f) -> p x f", x=1)
                            .to_broadcast([P, 3, H]),
                            qkv[:, :, 1, :])
                        nc.vector.reduce_sum(
                            scores[:, qt],
                            prod[:].rearrange("p k (h d) -> p k h d", d=HD),
                            axis=AX.X)
                    # softmax over kt (width 3) on a kt-innermost view.
                    # scores are bounded (|s| < ~30 for this model scale), so
                    # the max-subtraction is skipped: exp is safe in fp32.
                    sv = scores.rearrange("p q k h -> p q h k")
                    es = rpool.tile([P, 3, NH, 3], FP32, tag="es", bufs=2,
                                    name=f"es{st_}_{j}")
                    nc.scalar.activation(es[:], sv, AF.Exp)
                    sm = rpool.tile([P, 3 * NH], FP32, tag="mx", bufs=2,
                                    name=f"sm{st_}_{j}")
                    nc.vector.reduce_sum(sm[:], es[:], axis=AX.X)
                    rec = rpool.tile([P, 3 * NH], FP32, tag="mx", bufs=2,
                                     name=f"rc{st_}_{j}")
                    nc.vector.reciprocal(rec[:], sm[:])
                    nc.vector.tensor_mul(
                        es[:], es[:],
                        rec[:].rearrange("p (a h) -> p a h", h=NH)[:, :, :, None]
                        .to_broadcast([P, 3, NH, 3]))

                    # ctx[t, h] = sum_kt attn[t,h,kt] * v[kt,h]
                    ctx = rpool.tile([P, 3, H], BF16, tag="ctx", bufs=1,
                                     name=f"cx{st_}_{j}")
                    for t in range(3):
                        for h in range(NH):
                            blk = ctx[:, t, h * HD:(h + 1) * HD]
                            nc.scalar.mul(
                                blk, qkv[:, 0, 2, h * HD:(h + 1) * HD],
                                es[:, t, h, 0:1])
                            for kt in (1, 2):
                                nc.vector.scalar_tensor_tensor(
                                    out=blk,
                                    in0=qkv[:, kt, 2, h * HD:(h + 1) * HD],
                                    scalar=es[:, t, h, kt:kt + 1],
                                    in1=blk, op0=OP.mult, op1=OP.add)
                    for t in range(3):
                        nc.scalar.dma_start_transpose(
                            ctxT[t][:, j], ctx[:, t, :])

                # ---- stage 4: out_proj + residual + LN1 ----
                x_nat = apool.tile([P, 3, NB, H], BF16, tag="xnat", name=f"xn{st_}")
                xT = [apool.tile([P, NB, KH, P], BF16, tag="xT", bufs=1,
                                 name=f"xT{st_}_{t}") for t in range(3)]
                for t in range(3):
                    for j in range(NB):
                        ps = pspool.tile([P, H], FP32, tag="ps",
                                         name=f"ps_op{st_}_{t}_{j}")
                        nc.tensor.matmul(ps[:], ones_col[:], outb_row[:],
                                         start=True, stop=False)
                        for k in range(KH):
                            nc.tensor.matmul(
                                ps[:], ctxT[t][:, j, k, :],
                                owT[:, k, :], start=False, stop=False)
                        nc.tensor.matmul(ps[:], ident[:], comb_nat[:, t, j, :],
                                         start=False, stop=True)
                        layer_norm(ps, x_nat[:, t, j, :],
                                   f"l1_{st_}_{t}_{j}", ln1_affine, g1_bc, be1_bc)
                    nc.scalar.dma_start_transpose(
                        xT[t][:].rearrange("p j k b -> p (j k) b"), x_nat[:, t])

                # ---- stages 5+6: FFN + residual + LN2, pooled accumulate ----
                pooled = apool.tile([P, NB, H], BF16, tag="pooled", name=f"pl{st_}")
                for t in range(3):
                    hT = apool.tile([P, KF, ST], BF16, tag="hT", bufs=1,
                                    name=f"hT{st_}_{t}")
                    for c in range(KF):
                        ps = pspool.tile([P, NB, P], FP32, tag="ps",
                                         name=f"ps_f1{st_}_{t}_{c}")
                        for k in range(KH):
                            nc.tensor.matmul(
                                ps[:], w1T[:, k, c * P:(c + 1) * P],
                                xT[t][:, :, k, :], start=(k == 0),
                                stop=(k == KH - 1))
                        nc.scalar.activation(
                            hT[:, c, :],
                            ps[:].rearrange("p j b -> p (j b)"), AF.Relu,
                            bias=b1_pp[:, c:c + 1])
                    for j in range(NB):
                        ps = pspool.tile([P, H], FP32, tag="ps",
                                         name=f"ps_f2{st_}_{t}_{j}")
                        nc.tensor.matmul(ps[:], ones_col[:], b2_row[:],
                                         start=True, stop=False)
                        for k in range(KF):
                            nc.tensor.matmul(
                                ps[:], hT[:, k, j * P:(j + 1) * P],
                                w2T[:, k, :], start=False, stop=(k == KF - 1))
                        x2 = rpool.tile([P, H], BF16, tag="x2", bufs=2,
                                        name=f"x2_{st_}_{t}_{j}")
                        nc.vector.tensor_add(x2[:], ps[:], x_nat[:, t, j, :])
                        ps = x2
                        if t == 0:
                            layer_norm(ps, pooled[:, j, :],
                                       f"l2_{st_}_{t}_{j}", False, None, None)
                        else:
                            n2t = rpool.tile([P, H], BF16, tag="n2t", bufs=1,
                                             name=f"n2_{st_}_{t}_{j}")
                            layer_norm(ps, n2t[:],
                                       f"l2_{st_}_{t}_{j}", False, None, None)
                            nc.vector.scalar_tensor_tensor(
                                out=pooled[:, j, :], in0=n2t[:], scalar=1.0,
                                in1=pooled[:, j, :], op0=OP.bypass, op1=OP.add)

                # ---- stage 7: add pooled/3 into the early final partial ----
                for j in range(NB):
                    if not ln2_affine:
                        nc.vector.scalar_tensor_tensor(
                            out=outt[:, j, :], in0=pooled[:, j, :],
                            scalar=1.0 / 3, in1=outt[:, j, :],
                            op0=OP.mult, op1=OP.add)
                    else:
                        nsg = rpool.tile([P, H], BF16, tag="nsg", bufs=2,
                                         name=f"ng{st_}_{j}")
                        nc.vector.tensor_mul(nsg[:], pooled[:, j, :], g2_bc[:])
                        nc.vector.scalar_tensor_tensor(
                            out=outt[:, j, :], in0=nsg[:], scalar=1.0 / 3,
                            in1=outt[:, j, :], op0=OP.mult, op1=OP.add)
                nc.scalar.dma_start(
                    out_d[r0:r0 + ST].rearrange("(j p) f -> p j f", p=P),
                    outt[:])

            for mp in reversed(main_pools):
                mp.__exit__(None, None, None)

    nc.compile()
    return nc


_CACHE = {}


def _get_nc(b_core, ln1_affine, ln2_affine):
    key = (b_core, ln1_affine, ln2_affine)
    if key not in _CACHE:
        _CACHE[key] = build_nc(b_core, ln1_affine, ln2_affine)
    return _CACHE[key]


def kernel(**inputs):
    inp = {k: np.asarray(v, dtype=np.float32) for k, v in inputs.items()}
    ln1_affine = not (np.all(inp["g1"] == 1.0) and np.all(inp["be1"] == 0.0))
    ln2_affine = not np.all(inp["g2"] == 1.0)

    nc = _get_nc(B_CORE, ln1_affine, ln2_affine)

    shared = {k: inp[k] for k in inp
              if k not in ("visual_feat", "tactile_feat", "arm_feat")}
    in_maps = []
    for c in range(N_CORES):
        sl = slice(c * B_CORE, (c + 1) * B_CORE)
        m = dict(shared)
        m["visual_feat"] = np.ascontiguousarray(inp["visual_feat"][sl])
        m["tactile_feat"] = np.ascontiguousarray(inp["tactile_feat"][sl])
        m["arm_feat"] = np.ascontiguousarray(inp["arm_feat"][sl])
        in_maps.append(m)

    res = run_bass_kernel_spmd(nc, in_maps, core_ids=list(range(N_CORES)))
    return np.concatenate([res.results[c]["out"] for c in range(N_CORES)], axis=0)



# revision 23
# speedup vs baseline: 1.4327x; 1.4327x over previous
"""Trainium2 Bass kernel for nn_AttentionFusion (dense_transformer).

Pure data parallel across 8 NeuronCores: batch 16384 is split into 8 shards
of 2048 rows; weights (~7MB) are replicated on every core.  Each core runs
an identical fused program:

  proj(v,t,a) -> 3-token seq -> MHA (seq_len=3, 4 heads) -> +res -> LN ->
  FFN(relu) -> +res -> LN -> mean-pool over tokens -> + orig @ Wo.T + bo

Per-core design (512-row supertiles):
  - All matmuls on PE in bf16 with fp32 PSUM accumulation.  Activations are
    kept in natural layout [128 rows, feat] for VectorE/ScalarE work;
    feature-on-partition ("transposed") bf16 copies for matmul lhsT are
    produced by the DMA xbar transpose engine, one batched instruction per
    [128 x 512] (or wider) natural block using layout [P, j, k, 128].
  - FFN1 emits a transposed hidden so the per-feature bias+relu fuse into
    ScalarE's PSUM->SBUF copyback (bias is per-partition there).
  - Residual adds ride the PE via identity-matmul accumulation into PSUM;
    LayerNorm stats use VectorE bn_stats/bn_aggr straight off PSUM;
    rsqrt = exp(-0.5*ln(var+eps)) on ScalarE.
  - The activation-table registry is monkeypatched down to the single set
    `natural_log_exp_and_others` (contains ln/exp/identity/copy/relu) so
    ScalarE never reloads tables (~2.7us each) mid-kernel.
  - 1/sqrt(HD) is folded into the q projection weights/bias at init.
  - Constant bias vectors enter PSUM via K=1 matmuls against a ones column.
"""

import os
import sys

for _p in ("/opt/trn_rl_repo",):
    if _p not in sys.path and os.path.isdir(_p):
        sys.path.insert(0, _p)

import numpy as np

import concourse.bacc as bacc
import concourse.mybir as mybir
import concourse.tile as tile
from concourse.bass_utils import run_bass_kernel_spmd
from concourse.masks import make_identity

# Pin ScalarE to one activation-table set: keep only natural_log_exp_and_others
# populated (its index must stay aligned with act_info.json, so other sets are
# emptied rather than removed).
import concourse.hw_specs as _hw_specs

_ORIG_GET_TABLES = _hw_specs.get_activation_tables
_KEEP_SET = "natural_log_exp_and_others"


def _pinned_tables(module_arch):
    t = _ORIG_GET_TABLES(module_arch)
    if _KEEP_SET in t:
        t = {k: (v if k == _KEEP_SET else set()) for k, v in t.items()}
    return t


bacc.get_activation_tables = _pinned_tables

# Problem constants (hardcoded per harness contract).
B, H, NH, HD = 16384, 512, 4, 128
FF = 4 * H
EPS = 1e-5
N_CORES = 8
B_CORE = B // N_CORES  # 2048
P = 128
ST = 512               # supertile rows
NB = ST // P           # batch sub-tiles per supertile
KH = H // P            # k-chunks over hidden dim
KF = FF // P           # k-chunks over FF dim

FP32 = mybir.dt.float32
BF16 = mybir.dt.bfloat16
AX = mybir.AxisListType
OP = mybir.AluOpType
AF = mybir.ActivationFunctionType


def build_nc(b_core=B_CORE, ln1_affine=False, ln2_affine=False, passes=1):
    """Build the per-core Bass program."""
    nst = b_core // ST
    assert nst * ST == b_core

    nc = bacc.Bacc("TRN2", target_bir_lowering=False, debug=False, num_devices=1)

    # ---- DRAM I/O ----
    vf = nc.dram_tensor("visual_feat", (b_core, H), FP32, kind="ExternalInput")
    tf = nc.dram_tensor("tactile_feat", (b_core, H), FP32, kind="ExternalInput")
    af = nc.dram_tensor("arm_feat", (b_core, H), FP32, kind="ExternalInput")
    wd = {
        "Wv": nc.dram_tensor("Wv", (H, H), FP32, kind="ExternalInput"),
        "Wt": nc.dram_tensor("Wt", (H, H), FP32, kind="ExternalInput"),
        "Wa": nc.dram_tensor("Wa", (H, H), FP32, kind="ExternalInput"),
        "in_proj_w": nc.dram_tensor("in_proj_w", (3 * H, H), FP32, kind="ExternalInput"),
        "out_w": nc.dram_tensor("out_w", (H, H), FP32, kind="ExternalInput"),
        "w1": nc.dram_tensor("w1", (FF, H), FP32, kind="ExternalInput"),
        "w2": nc.dram_tensor("w2", (H, FF), FP32, kind="ExternalInput"),
        "Wo": nc.dram_tensor("Wo", (H, 3 * H), FP32, kind="ExternalInput"),
    }
    bd = {}
    for nm, sz in [("bv", H), ("bt", H), ("ba", H), ("in_proj_b", 3 * H),
                   ("out_b", H), ("b1", FF), ("b2", H), ("g1", H), ("be1", H),
                   ("g2", H), ("be2", H), ("bo", H)]:
        bd[nm] = nc.dram_tensor(nm, (sz,), FP32, kind="ExternalInput")
    out_d = nc.dram_tensor("out", (b_core, H), FP32, kind="ExternalOutput")

    feats = [vf, tf, af]
    inv_sqrt_hd = float(1.0 / np.sqrt(HD))

    with tile.TileContext(nc) as tc:
        with tc.tile_pool(name="const", bufs=1) as cpool, \
             tc.tile_pool(name="ps", bufs=8, space="PSUM") as pspool:
            # ================= init (wstage pool is recycled) =============
            with tc.tile_pool(name="wstage", bufs=1) as wpool:
                # ---- tiny constants & bias tiles (DMAs head the SP queue;
                # broadcasts ride a PE ones-matmul so GPSIMD stays free for
                # the weight casts) ----
                ones_col = cpool.tile([1, P], BF16, tag="ones", name="ones_col")
                nc.vector.memset(ones_col[:], 1.0)
                eps_pp = cpool.tile([P, 1], FP32, tag="eps", name="eps_pp")
                nc.vector.memset(eps_pp[:], EPS)
                ident = cpool.tile([P, P], BF16, tag="ident", name="ident")
                make_identity(nc, ident[:])

                def bcast_tile(name, src_ap, n, dt=BF16):
                    bt_ = cpool.tile([P, n], dt, tag=f"bc_{name}", name=f"{name}_bc")
                    for s0 in range(0, n, H):
                        row = wpool.tile([1, H], FP32, tag="brow", bufs=2,
                                         name=f"{name}_row{s0}")
                        nc.sync.dma_start(row[:], src_ap[None, s0:s0 + H])
                        rowc = wpool.tile([1, H], BF16, tag="browc", bufs=2,
                                          name=f"{name}_rowc{s0}")
                        nc.vector.tensor_copy(rowc[:], row[:])
                        bps = pspool.tile([P, H], FP32, tag="ps",
                                          name=f"{name}_ps{s0}")
                        nc.tensor.matmul(bps[:], ones_col[:], rowc[:],
                                         start=True, stop=True)
                        nc.scalar.copy(bt_[:, s0:s0 + H], bps[:])
                    return bt_

                bmod = [bcast_tile(nm, bd[nm], H) for nm in ("bv", "bt", "ba")]

                def brow_bf(name, src_ap, n, scale=None):
                    rowf = wpool.tile([1, n], FP32, tag="brow", bufs=2,
                                      name=f"{name}_rowf")
                    nc.sync.dma_start(rowf[:], src_ap)
                    rowb = cpool.tile([1, n], BF16, tag=f"br_{name}",
                                      name=f"{name}_rowb")
                    if scale is None:
                        nc.vector.tensor_copy(rowb[:], rowf[:])
                    else:
                        nc.vector.tensor_scalar_mul(rowb[:], rowf[:], scale)
                    return rowb

                ipbq_row = brow_bf("ipbq", bd["in_proj_b"][None, 0:H], H,
                                   scale=inv_sqrt_hd)
                ipbk_row = brow_bf("ipbk", bd["in_proj_b"][None, H:2 * H], H)
                ipbv_row = brow_bf("ipbv", bd["in_proj_b"][None, 2 * H:], H)
                ipb_rows = [ipbq_row, ipbk_row, ipbv_row]
                outb_row = brow_bf("outb", bd["out_b"][None, :], H)
                b2_row = brow_bf("b2", bd["b2"][None, :], H)

                # bo_eff = bo + be2 (be2 enters pooled exactly once post-mean)
                bo_rowf = wpool.tile([1, H], FP32, tag="brow", bufs=2,
                                     name="bo_rowf")
                nc.sync.dma_start(bo_rowf[:], bd["bo"][None, :])
                be2_rowf = wpool.tile([1, H], FP32, tag="brow", bufs=2,
                                      name="be2_rowf")
                nc.sync.dma_start(be2_rowf[:], bd["be2"][None, :])
                boe_row = cpool.tile([1, H], BF16, tag="br_boe", name="boe_rowb")
                nc.vector.tensor_add(boe_row[:], bo_rowf[:], be2_rowf[:])

                # b1 in per-partition layout [128, KF]
                b1_pp = cpool.tile([P, KF], FP32, tag="b1pp", name="b1_pp")
                nc.sync.dma_start(b1_pp[:], bd["b1"].rearrange("(c p) -> p c", p=P))

                g1_bc = be1_bc = g2_bc = None
                if ln1_affine:
                    g1_bc = bcast_tile("g1", bd["g1"], H)
                    be1_bc = bcast_tile("be1", bd["be1"], H)
                if ln2_affine:
                    g2_bc = bcast_tile("g2", bd["g2"], H)

                # ---- weights: PE-transpose (fp32) + ScalarE bf16 copyback.
                # The PE is idle during init and this keeps the DMA stream
                # copy-only (no xbar transpose<->copy mode serialization). ----
                ident_f = wpool.tile([P, P], FP32, tag="identf", name="ident_f")
                make_identity(nc, ident_f[:])

                def prep_weight(name, dram, n_out, n_in):
                    """W.T in [P(=in chunk), kc_n, n_out] bf16."""
                    oc_n = n_out // P
                    kc_n = n_in // P
                    nat = wpool.tile([P, oc_n, n_in], FP32, tag="wstage", bufs=2,
                                     name=f"{name}_nat")
                    nc.sync.dma_start(nat[:], dram.rearrange("(c p) f -> p c f", p=P))
                    wt = cpool.tile([P, kc_n, n_out], BF16, tag=f"wt_{name}",
                                    name=f"{name}_T")
                    for k in range(kc_n):
                        for og in range(0, n_out, 512):
                            gw = min(512, n_out - og)
                            tp = pspool.tile([P, 512], FP32, tag="ps",
                                             name=f"tp_{name}_{k}_{og}")
                            for oc in range(og // P, (og + gw) // P):
                                nc.tensor.transpose(
                                    tp[:, (oc * P - og):(oc * P - og) + P],
                                    nat[:, oc, k * P:(k + 1) * P],
                                    ident_f[:])
                            nc.scalar.copy(wt[:, k, og:og + gw], tp[:, :gw])
                    return wt

                # prep in PE-consumption order: proj, early-final, qkv,
                # out_proj, ffn1, ffn2
                wvT = prep_weight("Wv", wd["Wv"], H, H)
                wtT = prep_weight("Wt", wd["Wt"], H, H)
                waT = prep_weight("Wa", wd["Wa"], H, H)
                woT = prep_weight("wo", wd["Wo"], H, 3 * H)
                ipwT = prep_weight("ipw", wd["in_proj_w"], 3 * H, H)
                owT = prep_weight("ow", wd["out_w"], H, H)
                w1T = prep_weight("w1", wd["w1"], FF, H)
                w2T = prep_weight("w2", wd["w2"], H, FF)
                wTs = [wvT, wtT, waT]

                # q weights absorb the 1/sqrt(HD) score scale (GPSIMD: keeps
                # VectorE's stream free of weight-load dependencies)
                nc.gpsimd.tensor_scalar_mul(ipwT[:, :, 0:H], ipwT[:, :, 0:H],
                                            inv_sqrt_hd)

            main_pools = (
                tc.tile_pool(name="act", bufs=1),
                tc.tile_pool(name="rot", bufs=3),
            )
            apool = main_pools[0].__enter__()
            rpool = main_pools[1].__enter__()

            qkv_bufs = 1 if (ln1_affine or ln2_affine) else 2

            # ---- helpers ----
            def layer_norm(ps, dst, tagp, affine, g_bc, be_bc):
                """dst = LN(ps)[*g + be]; ps is PSUM fp32 holding x+residual."""
                bns = rpool.tile([P, 6], FP32, tag="lns6", bufs=4, name=f"b_{tagp}")
                nc.vector.bn_stats(bns[:], ps[:])
                bna = rpool.tile([P, 2], FP32, tag="lns2", bufs=3, name=f"a_{tagp}")
                nc.vector.bn_aggr(bna[:], bns[:])
                mean, var = bna[:, 0:1], bna[:, 1:2]
                lnv = rpool.tile([P, 1], FP32, tag="lns", bufs=6, name=f"lv_{tagp}")
                nc.scalar.activation(lnv[:], var, AF.Ln, bias=eps_pp[:])
                rstd = rpool.tile([P, 1], FP32, tag="lns", bufs=6, name=f"rs_{tagp}")
                nc.scalar.activation(rstd[:], lnv[:], AF.Exp, scale=-0.5)
                nmr = rpool.tile([P, 1], FP32, tag="lns", bufs=6, name=f"nm_{tagp}")
                nc.vector.tensor_scalar(
                    nmr[:], mean, rstd[:], -1.0, op0=OP.mult, op1=OP.mult)
                if not affine:
                    nc.scalar.activation(dst, ps[:], AF.Identity,
                                         bias=nmr[:], scale=rstd[:])
                else:
                    nrm = rpool.tile([P, H], BF16, tag="lnnrm", bufs=2,
                                     name=f"nr_{tagp}")
                    nc.scalar.activation(nrm[:], ps[:], AF.Identity,
                                         bias=nmr[:], scale=rstd[:])
                    nc.vector.tensor_mul(dst, nrm[:], g_bc[:])
                    if be_bc is not None:
                        nc.vector.tensor_add(dst, dst, be_bc[:])

            # ================= main loop =================
            # Transposed activations use layout [P, NB(j), KH(k), 128]:
            # ZT[p, j, k, b] = Z[j*128+b, k*128+p]; a whole [128, NB*H]
            # natural block transposes in ONE xbar instruction.
            for st_ in range(nst * passes):
                st = st_ % nst
                r0 = st * ST

                # ---- stage 0: feats load + cast + batched transpose ----
                featsT = []
                for m in range(3):
                    fT = apool.tile([P, NB, KH, P], BF16, tag=f"fT{m}",
                                    name=f"fT{st_}_{m}")
                    for j in range(NB):
                        fnat = rpool.tile([P, H], FP32, tag="fnat", bufs=2,
                                          name=f"fn{st_}_{m}_{j}")
                        nc.scalar.dma_start(
                            fnat[:], feats[m][r0 + j * P:r0 + (j + 1) * P, :])
                        fbf = rpool.tile([P, H], BF16, tag="fbf", bufs=2,
                                         name=f"fb{st_}_{m}_{j}")
                        nc.scalar.copy(fbf[:], fnat[:])
                        nc.scalar.dma_start_transpose(fT[:, j], fbf[:])
                    featsT.append(fT)

                # ---- stage 1: modality projections -> combined (natural) ----
                comb_nat = apool.tile([P, 3, NB, H], BF16, tag="combn",
                                      name=f"combn{st_}")
                for m in range(3):
                    for j in range(NB):
                        ps = pspool.tile([P, H], FP32, tag="ps",
                                         name=f"ps_pj{st_}_{m}_{j}")
                        for k in range(KH):
                            nc.tensor.matmul(
                                ps[:], featsT[m][:, j, k, :],
                                wTs[m][:, k, :], start=(k == 0), stop=(k == KH - 1))
                        nc.vector.tensor_add(comb_nat[:, m, j, :], ps[:], bmod[m][:])

                # early final projection: orig @ Wo.T + bo_eff -> outt
                # (frees featsT so the next supertile's feat prep can start)
                outt = apool.tile([P, NB, H], FP32, tag="outt", bufs=1,
                                  name=f"ot{st_}")
                for j in range(NB):
                    ps = pspool.tile([P, H], FP32, tag="ps", name=f"ps_fi{st_}_{j}")
                    nc.tensor.matmul(ps[:], ones_col[:], boe_row[:],
                                     start=True, stop=False)
                    for m in range(3):
                        for k in range(KH):
                            nc.tensor.matmul(
                                ps[:], featsT[m][:, j, k, :],
                                woT[:, m * KH + k, :], start=False,
                                stop=(m == 2 and k == KH - 1))
                    nc.scalar.copy(outt[:, j, :], ps[:])

                combT = []
                for t in range(3):
                    cT = apool.tile([P, NB, KH, P], BF16, tag="combT", bufs=3,
                                    name=f"combT{st_}_{t}")
                    nc.scalar.dma_start_transpose(
                        cT[:].rearrange("p j k b -> p (j k) b"), comb_nat[:, t])
                    combT.append(cT)

                # ---- stages 2+3: per batch sub-tile: qkv + attention ----
                ctxT = [apool.tile([P, NB, KH, P], BF16, tag="ctxT", bufs=3,
                                   name=f"ctxT{st_}_{t}") for t in range(3)]
                for j in range(NB):
                    qkv = apool.tile([P, 3, 3, H], BF16, tag="qkv", bufs=qkv_bufs,
                                     name=f"qkv{st_}_{j}")
                    for t in range(3):
                        pss = [pspool.tile([P, H], FP32, tag="ps",
                                           name=f"ps_qk{st_}_{t}_{j}_{s3}")
                               for s3 in range(3)]
                        for s3 in range(3):
                            nc.tensor.matmul(pss[s3][:], ones_col[:],
                                             ipb_rows[s3][:],
                                             start=True, stop=False)
                        for k in range(KH):
                            for s3 in range(3):
                                nc.tensor.matmul(
                                    pss[s3][:], combT[t][:, j, k, :],
                                    ipwT[:, k, s3 * H:(s3 + 1) * H],
                                    start=False, stop=(k == KH - 1))
                        for s3 in range(3):
                            nc.scalar.copy(qkv[:, t, s3], pss[s3][:])

                    # scores[b, qt, kt, h] = sum_d q*k (q pre-scaled)
                    scores = rpool.tile([P, 3, 3, NH], FP32, tag="scores", bufs=2,
                                        name=f"sc{st_}_{j}")
                    for qt in range(3):
                        prod = rpool.tile([P, 3, H], BF16, tag="prod", bufs=1,
                                          name=f"pr{st_}_{j}_{qt}")
                        nc.vector.tensor_mul(
                            prod[:],
                            qkv[:, qt, 0, :].rearrange("p (x f) -> p x f", x=1)
                            .to_broadcast([P, 3, H]),
                            qkv[:, :, 1, :])
                        nc.vector.reduce_sum(
                            scores[:, qt],
                            prod[:].rearrange("p k (h d) -> p k h d", d=HD),
                            axis=AX.X)
                    # softmax over kt (width 3) on a kt-innermost view.
                    # scores are bounded (|s| < ~30 for this model scale), so
                    # the max-subtraction is skipped: exp is safe in fp32.
                    sv = scores.rearrange("p q k h -> p q h k")
                    es = rpool.tile([P, 3, NH, 3], FP32, tag="es", bufs=2,
                                    name=f"es{st_}_{j}")
                    nc.scalar.activation(es[:], sv, AF.Exp)
                    sm = rpool.tile([P, 3 * NH], FP32, tag="mx", bufs=2,
                                    name=f"sm{st_}_{j}")
                    nc.vector.reduce_sum(sm[:], es[:], axis=AX.X)
                    rec = rpool.tile([P, 3 * NH], FP32, tag="mx", bufs=2,
                                     name=f"rc{st_}_{j}")
                    nc.vector.reciprocal(rec[:], sm[:])
                    nc.vector.tensor_mul(
                        es[:], es[:],
                        rec[:].rearrange("p (a h) -> p a h", h=NH)[:, :, :, None]
                        .to_broadcast([P, 3, NH, 3]))

                    # ctx[t, h] = sum_kt attn[t,h,kt] * v[kt,h]
                    ctx = rpool.tile([P, 3, H], BF16, tag="ctx", bufs=1,
                                     name=f"cx{st_}_{j}")
                    for t in range(3):
                        for h in range(NH):
                            blk = ctx[:, t, h * HD:(h + 1) * HD]
                            nc.scalar.mul(
                                blk, qkv[:, 0, 2, h * HD:(h + 1) * HD],
                                es[:, t, h, 0:1])
                            for kt in (1, 2):
                                nc.vector.scalar_tensor_tensor(
                                    out=blk,
                                    in0=qkv[:, kt, 2, h * HD:(h + 1) * HD],
                                    scalar=es[:, t, h, kt:kt + 1],
                                    in1=blk, op0=OP.mult, op1=OP.add)
                    for t in range(3):
                        nc.scalar.dma_start_transpose(
                            ctxT[t][:, j], ctx[:, t, :])

                # ---- stage 4: out_proj + residual + LN1 ----
                x_nat = apool.tile([P, 3, NB, H], BF16, tag="xnat", name=f"xn{st_}")
                xT = [apool.tile([P, NB, KH, P], BF16, tag="xT", bufs=1,
                                 name=f"xT{st_}_{t}") for t in range(3)]
                for t in range(3):
                    for j in range(NB):
                        ps = pspool.tile([P, H], FP32, tag="ps",
                                         name=f"ps_op{st_}_{t}_{j}")
                        nc.tensor.matmul(ps[:], ones_col[:], outb_row[:],
                                         start=True, stop=False)
                        for k in range(KH):
                            nc.tensor.matmul(
                                ps[:], ctxT[t][:, j, k, :],
                                owT[:, k, :], start=False, stop=False)
                        nc.tensor.matmul(ps[:], ident[:], comb_nat[:, t, j, :],
                                         start=False, stop=True)
                        layer_norm(ps, x_nat[:, t, j, :],
                                   f"l1_{st_}_{t}_{j}", ln1_affine, g1_bc, be1_bc)
                    nc.scalar.dma_start_transpose(
                        xT[t][:].rearrange("p j k b -> p (j k) b"), x_nat[:, t])

                # ---- stages 5+6: FFN + residual + LN2, pooled accumulate ----
                pooled = apool.tile([P, NB, H], BF16, tag="pooled", name=f"pl{st_}")
                for t in range(3):
                    hT = apool.tile([P, KF, ST], BF16, tag="hT", bufs=1,
                                    name=f"hT{st_}_{t}")
                    for c in range(KF):
                        ps = pspool.tile([P, NB, P], FP32, tag="ps",
                                         name=f"ps_f1{st_}_{t}_{c}")
                        for k in range(KH):
                            nc.tensor.matmul(
                                ps[:], w1T[:, k, c * P:(c + 1) * P],
                                xT[t][:, :, k, :], start=(k == 0),
                                stop=(k == KH - 1))
                        nc.scalar.activation(
                            hT[:, c, :],
                            ps[:].rearrange("p j b -> p (j b)"), AF.Relu,
                            bias=b1_pp[:, c:c + 1])
                    for j in range(NB):
                        ps = pspool.tile([P, H], FP32, tag="ps",
                                         name=f"ps_f2{st_}_{t}_{j}")
                        nc.tensor.matmul(ps[:], ones_col[:], b2_row[:],
                                         start=True, stop=False)
                        for k in range(KF):
                            nc.tensor.matmul(
                                ps[:], hT[:, k, j * P:(j + 1) * P],
                                w2T[:, k, :], start=False, stop=(k == KF - 1))
                        x2 = rpool.tile([P, H], BF16, tag="x2", bufs=2,
                                        name=f"x2_{st_}_{t}_{j}")
                        nc.vector.tensor_add(x2[:], ps[:], x_nat[:, t, j, :])
                        ps = x2
                        if t == 0:
                            layer_norm(ps, pooled[:, j, :],
                                       f"l2_{st_}_{t}_{j}", False, None, None)
                        else:
                            n2t = rpool.tile([P, H], BF16, tag="n2t", bufs=1,
                                             name=f"n2_{st_}_{t}_{j}")
                            layer_norm(ps, n2t[:],
                                       f"l2_{st_}_{t}_{j}", False, None, None)
                            nc.vector.scalar_tensor_tensor(
                                out=pooled[:, j, :], in0=n2t[:], scalar=1.0,
                                in1=pooled[:, j, :], op0=OP.bypass, op1=OP.add)

                # ---- stage 7: add pooled/3 into the early final partial ----
                for j in range(NB):
                    if not ln2_affine:
                        nc.vector.scalar_tensor_tensor(
                            out=outt[:, j, :], in0=pooled[:, j, :],
                            scalar=1.0 / 3, in1=outt[:, j, :],
                            op0=OP.mult, op1=OP.add)
                    else:
                        nsg = rpool.tile([P, H], BF16, tag="nsg", bufs=2,
                                         name=f"ng{st_}_{j}")
                        nc.vector.tensor_mul(nsg[:], pooled[:, j, :], g2_bc[:])
                        nc.vector.scalar_tensor_tensor(
                            out=outt[:, j, :], in0=nsg[:], scalar=1.0 / 3,
                            in1=outt[:, j, :], op0=OP.mult, op1=OP.add)
                nc.scalar.dma_start(
                    out_d[r0:r0 + ST].rearrange("(j p) f -> p j f", p=P),
                    outt[:])

            for mp in reversed(main_pools):
                mp.__exit__(None, None, None)

    nc.compile()
    return nc


def build_fast(b_core=B_CORE, passes=1):
    """Zero-bias / non-affine-LN fast path.

    Same dataflow as build_nc but the three big matmul blocks (qkv, FFN1,
    FFN2) run in fp8e4 DoubleRow perf mode (K=256 per instruction, 0.5
    cycles/out-row: 4x the bf16 PE throughput in the cost model).  Weights
    for those blocks are stored as fp8(W.T * 64); the 1/64 descale folds
    into the PSUM->SBUF copyback (Act scale / DVE tensor_scalar).  proj,
    out_proj and the final Wo matmul stay bf16 for accuracy (their error
    feeds the residual stream / output directly).

    All bias terms and LN affine params are assumed zero/identity (checked
    by kernel()); bias machinery is omitted entirely.

    Engine placement: DMA issue rides the SP queue (565ns/issue there vs
    667 on Act, and SP is otherwise idle); fp32->bf16 feat casts and
    bf16->fp8 transposed-activation casts ride GPSIMD (Pool); PSUM
    copybacks split between Act and DVE; LN rstd smalls are batched per
    (LN, token) instead of per tile.
    """
    nst = b_core // ST
    assert nst * ST == b_core
    S = 64.0                     # fp8 weight pre-scale
    inv_sqrt_hd = float(1.0 / np.sqrt(HD))

    nc = bacc.Bacc("TRN2", target_bir_lowering=False, debug=False, num_devices=1)

    vf = nc.dram_tensor("visual_feat", (b_core, H), FP32, kind="ExternalInput")
    tf = nc.dram_tensor("tactile_feat", (b_core, H), FP32, kind="ExternalInput")
    af = nc.dram_tensor("arm_feat", (b_core, H), FP32, kind="ExternalInput")
    wd = {
        "Wv": nc.dram_tensor("Wv", (H, H), FP32, kind="ExternalInput"),
        "Wt": nc.dram_tensor("Wt", (H, H), FP32, kind="ExternalInput"),
        "Wa": nc.dram_tensor("Wa", (H, H), FP32, kind="ExternalInput"),
        "in_proj_w": nc.dram_tensor("in_proj_w", (3 * H, H), FP32, kind="ExternalInput"),
        "out_w": nc.dram_tensor("out_w", (H, H), FP32, kind="ExternalInput"),
        "w1": nc.dram_tensor("w1", (FF, H), FP32, kind="ExternalInput"),
        "w2": nc.dram_tensor("w2", (H, FF), FP32, kind="ExternalInput"),
        "Wo": nc.dram_tensor("Wo", (H, 3 * H), FP32, kind="ExternalInput"),
    }
    for nm, sz in [("bv", H), ("bt", H), ("ba", H), ("in_proj_b", 3 * H),
                   ("out_b", H), ("b1", FF), ("b2", H), ("g1", H), ("be1", H),
                   ("g2", H), ("be2", H), ("bo", H)]:
        nc.dram_tensor(nm, (sz,), FP32, kind="ExternalInput")
    out_d = nc.dram_tensor("out", (b_core, H), FP32, kind="ExternalOutput")

    feats = [vf, tf, af]
    FP8 = mybir.dt.float8e4
    DRPM = mybir.MatmulPerfMode.DoubleRow

    with tile.TileContext(nc) as tc:
        with tc.tile_pool(name="const", bufs=1) as cpool, \
             tc.tile_pool(name="ps", bufs=5, space="PSUM") as pspool:
            with tc.tile_pool(name="wstage", bufs=1) as wpool:
                eps_pp = cpool.tile([P, 1], FP32, tag="eps", name="eps_pp")
                nc.vector.memset(eps_pp[:], EPS)
                ident = cpool.tile([P, P], BF16, tag="ident", name="ident")
                make_identity(nc, ident[:])

                def prep_weight(name, dram, n_out, n_in, dt=BF16, scale_fn=None):
                    """W.T in [P(=in chunk), kc_n, n_out] layout, dtype dt.

                    Route: chunked fp32 DMA load (Act queue) -> DVE cast to
                    bf16 (with fp8 range pre-scale folded in) -> DMA xbar
                    transpose -> (fp8 only) DVE cast to fp8.  No PE work at
                    all, so supertile-0 compute starts immediately; loads are
                    chunked per 128-row block because the cost model's
                    DMA_ENGINES device is a single serial resource and
                    monolithic multi-MB copies would block latency-critical
                    activation transposes."""
                    oc_n = n_out // P
                    kc_n = n_in // P
                    nat = wpool.tile([P, oc_n, n_in], FP32, tag="wstage", bufs=2,
                                     name=f"{name}_nat")
                    natb = wpool.tile([P, oc_n, n_in], BF16, tag="wstageb", bufs=2,
                                      name=f"{name}_natb")
                    drv = dram.rearrange("(c p) f -> p c f", p=P)
                    wtb_pool = cpool if dt == BF16 else wpool
                    wtb = wtb_pool.tile(
                        [P, kc_n, n_out], BF16,
                        tag=f"wt_{name}" if dt == BF16 else "wstageT",
                        name=f"{name}_Tb",
                        **({} if dt == BF16 else {"bufs": 2}))
                    for oc in range(oc_n):
                        nc.scalar.dma_start(nat[:, oc:oc + 1, :], drv[:, oc:oc + 1, :])
                        if scale_fn is None:
                            nc.vector.tensor_copy(natb[:, oc, :], nat[:, oc, :])
                        else:
                            nc.vector.tensor_scalar_mul(
                                natb[:, oc, :], nat[:, oc, :], scale_fn(oc * P))
                        # transpose of this 128-row block lands as column
                        # block oc of every k-chunk row of W.T
                        nc.scalar.dma_start_transpose(
                            wtb[:, :, oc * P:(oc + 1) * P], natb[:, oc, :])
                    if dt == BF16:
                        return wtb
                    wt = cpool.tile([P, kc_n, n_out], dt, tag=f"wt_{name}",
                                    name=f"{name}_T")
                    for k in range(kc_n):
                        nc.vector.tensor_copy(wt[:, k, :], wtb[:, k, :])
                    return wt

                wvT = prep_weight("Wv", wd["Wv"], H, H)
                wtT = prep_weight("Wt", wd["Wt"], H, H)
                waT = prep_weight("Wa", wd["Wa"], H, H)
                woT = prep_weight("wo", wd["Wo"], H, 3 * H)
                # q block (out rows < H) folds the 1/sqrt(HD) score scale
                ipw8 = prep_weight("ipw", wd["in_proj_w"], 3 * H, H, dt=FP8,
                                   scale_fn=lambda orow: S * inv_sqrt_hd if orow < H else S)
                owT = prep_weight("ow", wd["out_w"], H, H)
                w1T8 = prep_weight("w1", wd["w1"], FF, H, dt=FP8,
                                   scale_fn=lambda orow: S)
                w2T8 = prep_weight("w2", wd["w2"], H, FF, dt=FP8,
                                   scale_fn=lambda orow: S)
                wTs = [wvT, wtT, waT]

            main_pools = (
                tc.tile_pool(name="act", bufs=1),
                tc.tile_pool(name="rot", bufs=3),
            )
            apool = main_pools[0].__enter__()
            rpool = main_pools[1].__enter__()

            # ---- batched LN helper ----
            # stats per tile via bn_stats/bn_aggr into a shared [P, n, 2]
            # tile; one Ln + one Exp + one STT per batch computes rstd and
            # -mean*rstd for all n tiles at once.
            def ln_stats(ps_list, tagp):
                n = len(ps_list)
                bna = rpool.tile([P, n, 2], FP32, tag="lnagg", bufs=3,
                                 name=f"agg_{tagp}")
                for i, ps in enumerate(ps_list):
                    bns = rpool.tile([P, 6], FP32, tag="lns6", bufs=4,
                                     name=f"b_{tagp}_{i}")
                    nc.vector.bn_stats(bns[:], ps[:])
                    nc.vector.bn_aggr(bna[:, i, :], bns[:])
                lnv = rpool.tile([P, n], FP32, tag="lnv", bufs=3,
                                 name=f"lv_{tagp}")
                nc.scalar.activation(lnv[:], bna[:, :, 1], AF.Ln, bias=eps_pp[:])
                rstd = rpool.tile([P, n], FP32, tag="lnv", bufs=3,
                                  name=f"rs_{tagp}")
                nc.scalar.activation(rstd[:], lnv[:], AF.Exp, scale=-0.5)
                nmr = rpool.tile([P, n], FP32, tag="lnv", bufs=3,
                                 name=f"nm_{tagp}")
                nc.vector.scalar_tensor_tensor(
                    out=nmr[:], in0=bna[:, :, 0], scalar=-1.0, in1=rstd[:],
                    op0=OP.mult, op1=OP.mult)
                return rstd, nmr

            # ---- feats prefetch helpers (stage 0 of supertile st+1 is
            # emitted inside supertile st's attention loop so the SP/DMA/Pool
            # work overlaps PE/DVE compute) ----
            def alloc_featsT(st_n):
                return [apool.tile([P, NB, KH, P], BF16, tag=f"fT{m}",
                                   name=f"fT{st_n}_{m}") for m in range(3)]

            def emit_feat_block(fTs, st_n, j):
                r0n = (st_n % nst) * ST
                for m in range(3):
                    fnat = rpool.tile([P, H], FP32, tag="fnat", bufs=2,
                                      name=f"fn{st_n}_{m}_{j}")
                    nc.sync.dma_start(
                        fnat[:], feats[m][r0n + j * P:r0n + (j + 1) * P, :])
                    fbf = rpool.tile([P, H], BF16, tag="fbf", bufs=2,
                                     name=f"fb{st_n}_{m}_{j}")
                    nc.gpsimd.tensor_copy(fbf[:], fnat[:])
                    nc.sync.dma_start_transpose(fTs[m][:, j], fbf[:])

            # ================= main loop =================
            ipw8v = ipw8[:].rearrange("p (kk i) n -> p kk i n", kk=2)
            w1T8v = w1T8[:].rearrange("p (kk i) n -> p kk i n", kk=2)
            w2T8v = w2T8[:].rearrange("p (cc i) n -> p cc i n", cc=KF // 2)
            sc3 = [inv_sqrt_hd / S, 1.0 / S, 1.0 / S]

            def emit_front(st_, fTs):
                """proj + Wo + comb transpose/cast chain for supertile st_.

                Called one supertile EARLY (right after st_-1's j loop) so
                the comb->cT->fp8 chain runs on Act/DMA/Pool while PE grinds
                through st_-1's FFN, and qkv(st_) can start immediately
                after it."""
                comb_nat = apool.tile([P, 3, NB, H], BF16, tag="combn", bufs=2,
                                      name=f"combn{st_}")
                combT8 = [apool.tile([P, NB, KH, P], FP8, tag="combT8", bufs=3,
                                     name=f"combT8{st_}_{t}") for t in range(3)]
                for j in range(NB):
                    for m in range(3):
                        ps = pspool.tile([P, H], FP32, tag="ps",
                                         name=f"ps_pj{st_}_{m}_{j}")
                        for k in range(KH):
                            nc.tensor.matmul(
                                ps[:], fTs[m][:, j, k, :],
                                wTs[m][:, k, :], start=(k == 0), stop=(k == KH - 1))
                        nc.scalar.copy(comb_nat[:, m, j, :], ps[:])
                        cT = rpool.tile([P, KH, P], BF16, tag="cT", bufs=4,
                                        name=f"cT{st_}_{m}_{j}")
                        nc.scalar.dma_start_transpose(cT[:], comb_nat[:, m, j, :])
                        nc.gpsimd.tensor_copy(combT8[m][:, j], cT[:])
                # early final projection: orig @ Wo.T (frees fTs)
                outt = apool.tile([P, NB, H], FP32, tag="outt", bufs=2,
                                  name=f"ot{st_}")
                for j in range(NB):
                    ps = pspool.tile([P, H], FP32, tag="ps", name=f"ps_fi{st_}_{j}")
                    for m in range(3):
                        for k in range(KH):
                            nc.tensor.matmul(
                                ps[:], fTs[m][:, j, k, :],
                                woT[:, m * KH + k, :], start=(m == 0 and k == 0),
                                stop=(m == 2 and k == KH - 1))
                    nc.scalar.copy(outt[:, j, :], ps[:])
                return dict(comb_nat=comb_nat, combT8=combT8, outt=outt)

            def emit_mid(st_, F):
                """stages 2..4, software-pipelined over j:
                  PE:  qkv(0) qkv(1) op(0) qkv(2) op(1) qkv(3) op(2) op(3)
                  DVE: attn(0) attn(1) stats(0) attn(2) stats(1) ...
                so out_proj(j) finds ctxT(j) transposed, and attention(j+1)
                keeps DVE busy while LN1(j) waits on PE.  Also interleaves
                the feats prefetch for supertile st_+1."""
                comb_nat, combT8 = F["comb_nat"], F["combT8"]
                ctxT = [apool.tile([P, NB, KH, P], BF16, tag="ctxT", bufs=3,
                                   name=f"ctxT{st_}_{t}") for t in range(3)]
                x_nat = apool.tile([P, 3, NB, H], BF16, tag="xnat", name=f"xn{st_}")
                xT8 = [apool.tile([P, KH, NB, P], FP8, tag="xT8", bufs=3,
                                  name=f"xT8{st_}_{t}") for t in range(3)]

                def emit_qkv(j):
                    qkv = apool.tile([P, 3, 3, H], BF16, tag="qkv", bufs=2,
                                     name=f"qkv{st_}_{j}")
                    for t in range(3):
                        pss = [pspool.tile([P, H], FP32, tag="ps",
                                           name=f"ps_qk{st_}_{t}_{j}_{s3}")
                               for s3 in range(3)]
                        for kk in range(2):
                            for s3 in range(3):
                                nc.tensor.matmul(
                                    pss[s3][:],
                                    combT8[t][:, j, 2 * kk:2 * kk + 2, :],
                                    ipw8v[:, kk, :, s3 * H:(s3 + 1) * H],
                                    start=(kk == 0), stop=(kk == 1),
                                    perf_mode=DRPM)
                        for s3 in range(3):
                            # descale (and q's 1/sqrt(HD)) on copyback
                            nc.scalar.activation(
                                qkv[:, t, s3], pss[s3][:], AF.Copy,
                                scale=sc3[s3])
                    return qkv

                def emit_attention(j, qkv):
                    # scores[b, qt, kt, h]: batched mul, 2 fold-adds over d,
                    # then a short reduce (TensorReduce gets no 2x mode, so
                    # folding 128->32 in 2x-eligible adds is cheaper)
                    prod = rpool.tile([P, 9, NH, HD], BF16, tag="prod", bufs=1,
                                      name=f"pr{st_}_{j}")
                    nc.vector.tensor_mul(
                        prod[:].rearrange("p (q k) h d -> p q k (h d)", q=3),
                        qkv[:, :, 0, :].rearrange("p q (x f) -> p q x f", x=1)
                        .to_broadcast([P, 3, 3, H]),
                        qkv[:, :, 1, :].rearrange("p (x k) f -> p x k f", x=1)
                        .to_broadcast([P, 3, 3, H]))
                    nc.vector.tensor_add(prod[:, :, :, 0:HD // 2],
                                         prod[:, :, :, 0:HD // 2],
                                         prod[:, :, :, HD // 2:])
                    nc.vector.tensor_add(prod[:, :, :, 0:HD // 4],
                                         prod[:, :, :, 0:HD // 4],
                                         prod[:, :, :, HD // 4:HD // 2])
                    scores = rpool.tile([P, 3, 3, NH], BF16, tag="scores", bufs=2,
                                        name=f"sc{st_}_{j}")
                    with nc.allow_low_precision("scores bf16: |s|<30"):
                        nc.vector.reduce_sum(
                            scores[:].rearrange("p q k h -> p (q k) h"),
                            prod[:, :, :, 0:HD // 4], axis=AX.X)
                    # softmax over kt (width 3); bounded scores, no max-sub
                    es = rpool.tile([P, 3, NH, 3], BF16, tag="es", bufs=2,
                                    name=f"es{st_}_{j}")
                    nc.scalar.activation(
                        es[:], scores.rearrange("p q k h -> p q h k"), AF.Exp)
                    sm = rpool.tile([P, 3 * NH], BF16, tag="mx", bufs=2,
                                    name=f"sm{st_}_{j}")
                    with nc.allow_low_precision("softmax denom: 3 terms"):
                        nc.vector.reduce_sum(sm[:], es[:], axis=AX.X)
                    rec = rpool.tile([P, 3 * NH], BF16, tag="mx", bufs=2,
                                     name=f"rc{st_}_{j}")
                    with nc.allow_low_precision("softmax 1/denom in bf16"):
                        nc.vector.reciprocal(rec[:], sm[:])
                    attnb = rpool.tile([P, 3, NH, 3], BF16, tag="attnb", bufs=2,
                                       name=f"at{st_}_{j}")
                    nc.vector.tensor_mul(
                        attnb[:], es[:],
                        rec[:].rearrange("p (a h) -> p a h", h=NH)[:, :, :, None]
                        .to_broadcast([P, 3, NH, 3]))

                    # ctx[t] = sum_kt attn[t,h,kt] * v[kt,h,:]
                    ctx = rpool.tile([P, 3, H], BF16, tag="ctx", bufs=2,
                                     name=f"cx{st_}_{j}")
                    for t in range(3):
                        ctxp = rpool.tile([P, 3, H], BF16, tag="ctxp", bufs=1,
                                          name=f"cp{st_}_{j}_{t}")
                        nc.vector.tensor_mul(
                            ctxp[:].rearrange("p k (h d) -> p k h d", d=HD),
                            qkv[:, :, 2, :].rearrange("p k (h d) -> p k h d", d=HD),
                            attnb[:, t].rearrange("p h k -> p k h")[:, :, :, None]
                            .to_broadcast([P, 3, NH, HD]))
                        ctmp = rpool.tile([P, H], BF16, tag="ctmp", bufs=1,
                                          name=f"ct{st_}_{j}_{t}")
                        nc.vector.tensor_add(ctmp[:], ctxp[:, 0, :], ctxp[:, 1, :])
                        nc.vector.tensor_add(ctx[:, t, :], ctmp[:], ctxp[:, 2, :])
                        nc.sync.dma_start_transpose(
                            ctxT[t][:, j], ctx[:, t, :])

                def emit_op_ln1(j):
                    pss = []
                    for t in range(3):
                        ps = pspool.tile([P, H], FP32, tag="psop", bufs=3,
                                         name=f"ps_op{st_}_{t}_{j}")
                        for k in range(KH):
                            nc.tensor.matmul(
                                ps[:], ctxT[t][:, j, k, :],
                                owT[:, k, :], start=(k == 0), stop=False)
                        nc.tensor.matmul(ps[:], ident[:], comb_nat[:, t, j, :],
                                         start=False, stop=True)
                        pss.append(ps)
                    rstd, nmr = ln_stats(pss, f"l1_{st_}_{j}")
                    for t in range(3):
                        nc.scalar.activation(x_nat[:, t, j, :], pss[t][:],
                                             AF.Identity, bias=nmr[:, t:t + 1],
                                             scale=rstd[:, t:t + 1])
                        xTs = rpool.tile([P, KH, P], BF16, tag="xTs", bufs=4,
                                         name=f"xTs{st_}_{t}_{j}")
                        nc.scalar.dma_start_transpose(xTs[:], x_nat[:, t, j, :])
                        nc.gpsimd.tensor_copy(xT8[t][:, :, j, :], xTs[:])

                featsT_next = None
                qkv_tiles = [None] * NB
                qkv_tiles[0] = emit_qkv(0)
                emit_attention(0, qkv_tiles[0])
                for j in range(NB):
                    if j + 1 < NB:
                        qkv_tiles[j + 1] = emit_qkv(j + 1)
                        emit_attention(j + 1, qkv_tiles[j + 1])
                    if st_ + 1 < n_st:
                        if j == 0:
                            featsT_next = alloc_featsT(st_ + 1)
                        emit_feat_block(featsT_next, st_ + 1, j)
                    emit_op_ln1(j)
                return dict(x_nat=x_nat, xT8=xT8, featsT_next=featsT_next)

            def emit_back(st_, F, M):
                """stages 5+6: FFN (fp8 DR) + residual + LN2, pooled; then
                stage 7 and the output store."""
                r0 = (st_ % nst) * ST
                x_nat, xT8, outt = M["x_nat"], M["xT8"], F["outt"]
                pooled = apool.tile([P, NB, H], BF16, tag="pooled", name=f"pl{st_}")
                for t in range(3):
                    hT8 = apool.tile([P, KF, ST], FP8, tag="hT8", bufs=1,
                                     name=f"hT8{st_}_{t}")
                    for c in range(KF):
                        ps = pspool.tile([P, NB, P], FP32, tag="ps",
                                         name=f"ps_f1{st_}_{t}_{c}")
                        for kk in range(2):
                            nc.tensor.matmul(
                                ps[:], w1T8v[:, kk, :, c * P:(c + 1) * P],
                                xT8[t][:, 2 * kk:2 * kk + 2, :, :]
                                .rearrange("p k j b -> p k (j b)"),
                                start=(kk == 0), stop=(kk == 1), perf_mode=DRPM)
                        # descale + relu on copyback, split Act/DVE
                        psv = ps[:].rearrange("p j b -> p (j b)")
                        if c % 2 == 0:
                            nc.scalar.activation(hT8[:, c, :], psv, AF.Relu,
                                                 scale=1.0 / S)
                        else:
                            nc.vector.tensor_scalar(
                                hT8[:, c, :], psv, 1.0 / S, 0.0,
                                op0=OP.mult, op1=OP.max)
                    x2s = []
                    for j in range(NB):
                        ps = pspool.tile([P, H], FP32, tag="ps",
                                         name=f"ps_f2{st_}_{t}_{j}")
                        for cc in range(KF // 2):
                            nc.tensor.matmul(
                                ps[:], hT8[:, 2 * cc:2 * cc + 2, j * P:(j + 1) * P],
                                w2T8v[:, cc, :, :],
                                start=(cc == 0), stop=(cc == KF // 2 - 1),
                                perf_mode=DRPM)
                        x2 = rpool.tile([P, H], BF16, tag="x2", bufs=4,
                                        name=f"x2_{st_}_{t}_{j}")
                        nc.vector.scalar_tensor_tensor(
                            out=x2[:], in0=ps[:], scalar=1.0 / S,
                            in1=x_nat[:, t, j, :], op0=OP.mult, op1=OP.add)
                        x2s.append(x2)
                    rstd, nmr = ln_stats(x2s, f"l2_{st_}_{t}")
                    for j in range(NB):
                        if t == 0:
                            nc.scalar.activation(pooled[:, j, :], x2s[j][:],
                                                 AF.Identity, bias=nmr[:, j:j + 1],
                                                 scale=rstd[:, j:j + 1])
                        else:
                            n2t = rpool.tile([P, H], BF16, tag="n2t", bufs=1,
                                             name=f"n2_{st_}_{t}_{j}")
                            nc.scalar.activation(n2t[:], x2s[j][:],
                                                 AF.Identity, bias=nmr[:, j:j + 1],
                                                 scale=rstd[:, j:j + 1])
                            nc.vector.tensor_add(pooled[:, j, :],
                                                 pooled[:, j, :], n2t[:])

                # stage 7: add pooled/3 into the early final partial
                for j in range(NB):
                    nc.vector.scalar_tensor_tensor(
                        out=outt[:, j, :], in0=pooled[:, j, :],
                        scalar=1.0 / 3, in1=outt[:, j, :],
                        op0=OP.mult, op1=OP.add)
                nc.sync.dma_start(
                    out_d[r0:r0 + ST].rearrange("(j p) f -> p j f", p=P),
                    outt[:])

            n_st = nst * passes
            featsT = alloc_featsT(0)
            for j in range(NB):
                emit_feat_block(featsT, 0, j)
            F = emit_front(0, featsT)
            for st_ in range(n_st):
                M = emit_mid(st_, F)
                if st_ + 1 < n_st:
                    F_next = emit_front(st_ + 1, M["featsT_next"])
                emit_back(st_, F, M)
                if st_ + 1 < n_st:
                    F = F_next

            for mp in reversed(main_pools):
                mp.__exit__(None, None, None)

    nc.compile()
    return nc


def build_fast(b_core=B_CORE, passes=1):
    """Zero-bias / non-affine-LN fast path.

    Same dataflow as build_nc but the three big matmul blocks (qkv, FFN1,
    FFN2) run in fp8e4 DoubleRow perf mode (K=256 per instruction, 0.5
    cycles/out-row: 4x the bf16 PE throughput in the cost model).  Weights
    for those blocks are stored as fp8(W.T * 64); the 1/64 descale folds
    into the PSUM->SBUF copyback (Act scale / DVE tensor_scalar).  proj,
    out_proj and the final Wo matmul stay bf16 for accuracy (their error
    feeds the residual stream / output directly).

    All bias terms and LN affine params are assumed zero/identity (checked
    by kernel()); bias machinery is omitted entirely.

    Engine placement: DMA issue rides the SP queue (565ns/issue there vs
    667 on Act, and SP is otherwise idle); fp32->bf16 feat casts and
    bf16->fp8 transposed-activation casts ride GPSIMD (Pool); PSUM
    copybacks split between Act and DVE; LN rstd smalls are batched per
    (LN, token) instead of per tile.
    """
    nst = b_core // ST
    assert nst * ST == b_core
    S = 64.0                     # fp8 weight pre-scale
    inv_sqrt_hd = float(1.0 / np.sqrt(HD))

    nc = bacc.Bacc("TRN2", target_bir_lowering=False, debug=False, num_devices=1)

    vf = nc.dram_tensor("visual_feat", (b_core, H), FP32, kind="ExternalInput")
    tf = nc.dram_tensor("tactile_feat", (b_core, H), FP32, kind="ExternalInput")
    af = nc.dram_tensor("arm_feat", (b_core, H), FP32, kind="ExternalInput")
    wd = {
        "Wv": nc.dram_tensor("Wv", (H, H), FP32, kind="ExternalInput"),
        "Wt": nc.dram_tensor("Wt", (H, H), FP32, kind="ExternalInput"),
        "Wa": nc.dram_tensor("Wa", (H, H), FP32, kind="ExternalInput"),
        "in_proj_w": nc.dram_tensor("in_proj_w", (3 * H, H), FP32, kind="ExternalInput"),
        "out_w": nc.dram_tensor("out_w", (H, H), FP32, kind="ExternalInput"),
        "w1": nc.dram_tensor("w1", (FF, H), FP32, kind="ExternalInput"),
        "w2": nc.dram_tensor("w2", (H, FF), FP32, kind="ExternalInput"),
        "Wo": nc.dram_tensor("Wo", (H, 3 * H), FP32, kind="ExternalInput"),
    }
    for nm, sz in [("bv", H), ("bt", H), ("ba", H), ("in_proj_b", 3 * H),
                   ("out_b", H), ("b1", FF), ("b2", H), ("g1", H), ("be1", H),
                   ("g2", H), ("be2", H), ("bo", H)]:
        nc.dram_tensor(nm, (sz,), FP32, kind="ExternalInput")
    out_d = nc.dram_tensor("out", (b_core, H), FP32, kind="ExternalOutput")

    feats = [vf, tf, af]
    FP8 = mybir.dt.float8e4
    DRPM = mybir.MatmulPerfMode.DoubleRow

    with tile.TileContext(nc) as tc:
        with tc.tile_pool(name="const", bufs=1) as cpool, \
             tc.tile_pool(name="ps", bufs=5, space="PSUM") as pspool:
            with tc.tile_pool(name="wstage", bufs=1) as wpool:
                eps_pp = cpool.tile([P, 1], FP32, tag="eps", name="eps_pp")
                nc.vector.memset(eps_pp[:], EPS)
                ident = cpool.tile([P, P], BF16, tag="ident", name="ident")
                make_identity(nc, ident[:])

                def prep_weight(name, dram, n_out, n_in, dt=BF16, scale_fn=None):
                    """W.T in [P(=in chunk), kc_n, n_out] layout, dtype dt.

                    Route: chunked fp32 DMA load (Act queue) -> DVE cast to
                    bf16 (with fp8 range pre-scale folded in) -> DMA xbar
                    transpose -> (fp8 only) DVE cast to fp8.  No PE work at
                    all, so supertile-0 compute starts immediately; loads are
                    chunked per 128-row block because the cost model's
                    DMA_ENGINES device is a single serial resource and
                    monolithic multi-MB copies would block latency-critical
                    activation transposes."""
                    oc_n = n_out // P
                    kc_n = n_in // P
                    nat = wpool.tile([P, oc_n, n_in], FP32, tag="wstage", bufs=2,
                                     name=f"{name}_nat")
                    natb = wpool.tile([P, oc_n, n_in], BF16, tag="wstageb", bufs=2,
                                      name=f"{name}_natb")
                    drv = dram.rearrange("(c p) f -> p c f", p=P)
                    wtb_pool = cpool if dt == BF16 else wpool
                    wtb = wtb_pool.tile(
                        [P, kc_n, n_out], BF16,
                        tag=f"wt_{name}" if dt == BF16 else "wstageT",
                        name=f"{name}_Tb",
                        **({} if dt == BF16 else {"bufs": 2}))
                    for oc in range(oc_n):
                        nc.scalar.dma_start(nat[:, oc:oc + 1, :], drv[:, oc:oc + 1, :])
                        if scale_fn is None:
                            nc.vector.tensor_copy(natb[:, oc, :], nat[:, oc, :])
                        else:
                            nc.vector.tensor_scalar_mul(
                                natb[:, oc, :], nat[:, oc, :], scale_fn(oc * P))
                        # transpose of this 128-row block lands as column
                        # block oc of every k-chunk row of W.T
                        nc.scalar.dma_start_transpose(
                            wtb[:, :, oc * P:(oc + 1) * P], natb[:, oc, :])
                    if dt == BF16:
                        return wtb
                    wt = cpool.tile([P, kc_n, n_out], dt, tag=f"wt_{name}",
                                    name=f"{name}_T")
                    for k in range(kc_n):
                        nc.vector.tensor_copy(wt[:, k, :], wtb[:, k, :])
                    return wt

                wvT = prep_weight("Wv", wd["Wv"], H, H)
                wtT = prep_weight("Wt", wd["Wt"], H, H)
                waT = prep_weight("Wa", wd["Wa"], H, H)
                woT = prep_weight("wo", wd["Wo"], H, 3 * H)
                # q block (out rows < H) folds the 1/sqrt(HD) score scale
                ipw8 = prep_weight("ipw", wd["in_proj_w"], 3 * H, H, dt=FP8,
                                   scale_fn=lambda orow: S * inv_sqrt_hd if orow < H else S)
                owT = prep_weight("ow", wd["out_w"], H, H)
                w1T8 = prep_weight("w1", wd["w1"], FF, H, dt=FP8,
                                   scale_fn=lambda orow: S)
                w2T8 = prep_weight("w2", wd["w2"], H, FF, dt=FP8,
                                   scale_fn=lambda orow: S)
                wTs = [wvT, wtT, waT]

            main_pools = (
                tc.tile_pool(name="act", bufs=1),
                tc.tile_pool(name="rot", bufs=3),
            )
            apool = main_pools[0].__enter__()
            rpool = main_pools[1].__enter__()

            # ---- batched LN helper ----
            # stats per tile via bn_stats/bn_aggr into a shared [P, n, 2]
            # tile; one Ln + one Exp + one STT per batch computes rstd and
            # -mean*rstd for all n tiles at once.
            def ln_stats(ps_list, tagp):
                n = len(ps_list)
                bna = rpool.tile([P, n, 2], FP32, tag="lnagg", bufs=3,
                                 name=f"agg_{tagp}")
                for i, ps in enumerate(ps_list):
                    bns = rpool.tile([P, 6], FP32, tag="lns6", bufs=4,
                                     name=f"b_{tagp}_{i}")
                    nc.vector.bn_stats(bns[:], ps[:])
                    nc.vector.bn_aggr(bna[:, i, :], bns[:])
                lnv = rpool.tile([P, n], FP32, tag="lnv", bufs=3,
                                 name=f"lv_{tagp}")
                nc.scalar.activation(lnv[:], bna[:, :, 1], AF.Ln, bias=eps_pp[:])
                rstd = rpool.tile([P, n], FP32, tag="lnv", bufs=3,
                                  name=f"rs_{tagp}")
                nc.scalar.activation(rstd[:], lnv[:], AF.Exp, scale=-0.5)
                nmr = rpool.tile([P, n], FP32, tag="lnv", bufs=3,
                                 name=f"nm_{tagp}")
                nc.vector.scalar_tensor_tensor(
                    out=nmr[:], in0=bna[:, :, 0], scalar=-1.0, in1=rstd[:],
                    op0=OP.mult, op1=OP.mult)
                return rstd, nmr

            # ---- feats prefetch helpers (stage 0 of supertile st+1 is
            # emitted inside supertile st's attention loop so the SP/DMA/Pool
            # work overlaps PE/DVE compute) ----
            def alloc_featsT(st_n):
                return [apool.tile([P, NB, KH, P], BF16, tag=f"fT{m}",
                                   name=f"fT{st_n}_{m}") for m in range(3)]

            def emit_feat_block(fTs, st_n, j):
                r0n = (st_n % nst) * ST
                for m in range(3):
                    fnat = rpool.tile([P, H], FP32, tag="fnat", bufs=2,
                                      name=f"fn{st_n}_{m}_{j}")
                    nc.sync.dma_start(
                        fnat[:], feats[m][r0n + j * P:r0n + (j + 1) * P, :])
                    fbf = rpool.tile([P, H], BF16, tag="fbf", bufs=2,
                                     name=f"fb{st_n}_{m}_{j}")
                    nc.gpsimd.tensor_copy(fbf[:], fnat[:])
                    nc.sync.dma_start_transpose(fTs[m][:, j], fbf[:])

            # ================= main loop =================
            ipw8v = ipw8[:].rearrange("p (kk i) n -> p kk i n", kk=2)
            w1T8v = w1T8[:].rearrange("p (kk i) n -> p kk i n", kk=2)
            w2T8v = w2T8[:].rearrange("p (cc i) n -> p cc i n", cc=KF // 2)
            sc3 = [inv_sqrt_hd / S, 1.0 / S, 1.0 / S]

            n_st = nst * passes
            featsT = alloc_featsT(0)
            for j in range(NB):
                emit_feat_block(featsT, 0, j)

            for st_ in range(n_st):
                st = st_ % nst
                r0 = st * ST

                # ---- stage 1: projections (bf16), j-outer so the first
                # (t, j=0) transposed fp8 blocks are ready asap ----
                comb_nat = apool.tile([P, 3, NB, H], BF16, tag="combn", bufs=2,
                                      name=f"combn{st_}")
                combT8 = [apool.tile([P, NB, KH, P], FP8, tag="combT8", bufs=3,
                                     name=f"combT8{st_}_{t}") for t in range(3)]
                for j in range(NB):
                    for m in range(3):
                        ps = pspool.tile([P, H], FP32, tag="ps",
                                         name=f"ps_pj{st_}_{m}_{j}")
                        for k in range(KH):
                            nc.tensor.matmul(
                                ps[:], featsT[m][:, j, k, :],
                                wTs[m][:, k, :], start=(k == 0), stop=(k == KH - 1))
                        nc.scalar.copy(comb_nat[:, m, j, :], ps[:])
                        cT = rpool.tile([P, KH, P], BF16, tag="cT", bufs=4,
                                        name=f"cT{st_}_{m}_{j}")
                        nc.scalar.dma_start_transpose(cT[:], comb_nat[:, m, j, :])
                        nc.gpsimd.tensor_copy(combT8[m][:, j], cT[:])

                # early final projection: orig @ Wo.T (frees featsT)
                outt = apool.tile([P, NB, H], FP32, tag="outt", bufs=1,
                                  name=f"ot{st_}")
                for j in range(NB):
                    ps = pspool.tile([P, H], FP32, tag="ps", name=f"ps_fi{st_}_{j}")
                    for m in range(3):
                        for k in range(KH):
                            nc.tensor.matmul(
                                ps[:], featsT[m][:, j, k, :],
                                woT[:, m * KH + k, :], start=(m == 0 and k == 0),
                                stop=(m == 2 and k == KH - 1))
                    nc.scalar.copy(outt[:, j, :], ps[:])

                # ---- stages 2..4, software-pipelined over j:
                #   PE:  qkv(0) qkv(1) op(0) qkv(2) op(1) qkv(3) op(2) op(3)
                #   DVE: attn(0) attn(1) stats(0) attn(2) stats(1) ...
                # so out_proj(j) finds ctxT(j) transposed, and attention(j+1)
                # keeps DVE busy while LN1(j) waits on PE. ----
                ctxT = [apool.tile([P, NB, KH, P], BF16, tag="ctxT", bufs=3,
                                   name=f"ctxT{st_}_{t}") for t in range(3)]
                x_nat = apool.tile([P, 3, NB, H], BF16, tag="xnat", name=f"xn{st_}")
                xT8 = [apool.tile([P, KH, NB, P], FP8, tag="xT8", bufs=3,
                                  name=f"xT8{st_}_{t}") for t in range(3)]

                def emit_qkv(j):
                    qkv = apool.tile([P, 3, 3, H], BF16, tag="qkv", bufs=2,
                                     name=f"qkv{st_}_{j}")
                    for t in range(3):
                        pss = [pspool.tile([P, H], FP32, tag="ps",
                                           name=f"ps_qk{st_}_{t}_{j}_{s3}")
                               for s3 in range(3)]
                        for kk in range(2):
                            for s3 in range(3):
                                nc.tensor.matmul(
                                    pss[s3][:],
                                    combT8[t][:, j, 2 * kk:2 * kk + 2, :],
                                    ipw8v[:, kk, :, s3 * H:(s3 + 1) * H],
                                    start=(kk == 0), stop=(kk == 1),
                                    perf_mode=DRPM)
                        for s3 in range(3):
                            # descale (and q's 1/sqrt(HD)) on copyback
                            nc.scalar.activation(
                                qkv[:, t, s3], pss[s3][:], AF.Copy,
                                scale=sc3[s3])
                    return qkv

                def emit_attention(j, qkv):
                    # scores[b, qt, kt, h]: batched mul, 2 fold-adds over d,
                    # then a short reduce (TensorReduce gets no 2x mode, so
                    # folding 128->32 in 2x-eligible adds is cheaper)
                    prod = rpool.tile([P, 9, NH, HD], BF16, tag="prod", bufs=1,
                                      name=f"pr{st_}_{j}")
                    nc.vector.tensor_mul(
                        prod[:].rearrange("p (q k) h d -> p q k (h d)", q=3),
                        qkv[:, :, 0, :].rearrange("p q (x f) -> p q x f", x=1)
                        .to_broadcast([P, 3, 3, H]),
                        qkv[:, :, 1, :].rearrange("p (x k) f -> p x k f", x=1)
                        .to_broadcast([P, 3, 3, H]))
                    nc.vector.tensor_add(prod[:, :, :, 0:HD // 2],
                                         prod[:, :, :, 0:HD // 2],
                                         prod[:, :, :, HD // 2:])
                    nc.vector.tensor_add(prod[:, :, :, 0:HD // 4],
                                         prod[:, :, :, 0:HD // 4],
                                         prod[:, :, :, HD // 4:HD // 2])
                    scores = rpool.tile([P, 3, 3, NH], BF16, tag="scores", bufs=2,
                                        name=f"sc{st_}_{j}")
                    with nc.allow_low_precision("scores bf16: |s|<30"):
                        nc.vector.reduce_sum(
                            scores[:].rearrange("p q k h -> p (q k) h"),
                            prod[:, :, :, 0:HD // 4], axis=AX.X)
                    # softmax over kt (width 3); bounded scores, no max-sub
                    es = rpool.tile([P, 3, NH, 3], BF16, tag="es", bufs=2,
                                    name=f"es{st_}_{j}")
                    nc.scalar.activation(
                        es[:], scores.rearrange("p q k h -> p q h k"), AF.Exp)
                    sm = rpool.tile([P, 3 * NH], BF16, tag="mx", bufs=2,
                                    name=f"sm{st_}_{j}")
                    with nc.allow_low_precision("softmax denom: 3 terms"):
                        nc.vector.reduce_sum(sm[:], es[:], axis=AX.X)
                    rec = rpool.tile([P, 3 * NH], BF16, tag="mx", bufs=2,
                                     name=f"rc{st_}_{j}")
                    with nc.allow_low_precision("softmax 1/denom in bf16"):
                        nc.vector.reciprocal(rec[:], sm[:])
                    attnb = rpool.tile([P, 3, NH, 3], BF16, tag="attnb", bufs=2,
                                       name=f"at{st_}_{j}")
                    nc.vector.tensor_mul(
                        attnb[:], es[:],
                        rec[:].rearrange("p (a h) -> p a h", h=NH)[:, :, :, None]
                        .to_broadcast([P, 3, NH, 3]))

                    # ctx[t] = sum_kt attn[t,h,kt] * v[kt,h,:]
                    ctx = rpool.tile([P, 3, H], BF16, tag="ctx", bufs=2,
                                     name=f"cx{st_}_{j}")
                    for t in range(3):
                        ctxp = rpool.tile([P, 3, H], BF16, tag="ctxp", bufs=1,
                                          name=f"cp{st_}_{j}_{t}")
                        nc.vector.tensor_mul(
                            ctxp[:].rearrange("p k (h d) -> p k h d", d=HD),
                            qkv[:, :, 2, :].rearrange("p k (h d) -> p k h d", d=HD),
                            attnb[:, t].rearrange("p h k -> p k h")[:, :, :, None]
                            .to_broadcast([P, 3, NH, HD]))
                        ctmp = rpool.tile([P, H], BF16, tag="ctmp", bufs=1,
                                          name=f"ct{st_}_{j}_{t}")
                        nc.vector.tensor_add(ctmp[:], ctxp[:, 0, :], ctxp[:, 1, :])
                        nc.vector.tensor_add(ctx[:, t, :], ctmp[:], ctxp[:, 2, :])
                        nc.sync.dma_start_transpose(
                            ctxT[t][:, j], ctx[:, t, :])

                def emit_op_ln1(j):
                    pss = []
                    for t in range(3):
                        ps = pspool.tile([P, H], FP32, tag="psop", bufs=3,
                                         name=f"ps_op{st_}_{t}_{j}")
                        for k in range(KH):
                            nc.tensor.matmul(
                                ps[:], ctxT[t][:, j, k, :],
                                owT[:, k, :], start=(k == 0), stop=False)
                        nc.tensor.matmul(ps[:], ident[:], comb_nat[:, t, j, :],
                                         start=False, stop=True)
                        pss.append(ps)
                    rstd, nmr = ln_stats(pss, f"l1_{st_}_{j}")
                    for t in range(3):
                        nc.scalar.activation(x_nat[:, t, j, :], pss[t][:],
                                             AF.Identity, bias=nmr[:, t:t + 1],
                                             scale=rstd[:, t:t + 1])
                        xTs = rpool.tile([P, KH, P], BF16, tag="xTs", bufs=4,
                                         name=f"xTs{st_}_{t}_{j}")
                        nc.scalar.dma_start_transpose(xTs[:], x_nat[:, t, j, :])
                        nc.gpsimd.tensor_copy(xT8[t][:, :, j, :], xTs[:])

                qkv_tiles = [None] * NB
                qkv_tiles[0] = emit_qkv(0)
                emit_attention(0, qkv_tiles[0])
                for j in range(NB):
                    if j + 1 < NB:
                        qkv_tiles[j + 1] = emit_qkv(j + 1)
                        emit_attention(j + 1, qkv_tiles[j + 1])
                    if st_ + 1 < n_st:
                        if j == 0:
                            featsT_next = alloc_featsT(st_ + 1)
                        emit_feat_block(featsT_next, st_ + 1, j)
                    emit_op_ln1(j)

                # ---- stages 5+6: FFN (fp8 DR) + residual + LN2, pooled ----
                pooled = apool.tile([P, NB, H], BF16, tag="pooled", name=f"pl{st_}")
                for t in range(3):
                    hT8 = apool.tile([P, KF, ST], FP8, tag="hT8", bufs=1,
                                     name=f"hT8{st_}_{t}")
                    for c in range(KF):
                        ps = pspool.tile([P, NB, P], FP32, tag="ps",
                                         name=f"ps_f1{st_}_{t}_{c}")
                        for kk in range(2):
                            nc.tensor.matmul(
                                ps[:], w1T8v[:, kk, :, c * P:(c + 1) * P],
                                xT8[t][:, 2 * kk:2 * kk + 2, :, :]
                                .rearrange("p k j b -> p k (j b)"),
                                start=(kk == 0), stop=(kk == 1), perf_mode=DRPM)
                        # descale + relu on copyback, split Act/DVE
                        psv = ps[:].rearrange("p j b -> p (j b)")
                        if c % 2 == 0:
                            nc.scalar.activation(hT8[:, c, :], psv, AF.Relu,
                                                 scale=1.0 / S)
                        else:
                            nc.vector.tensor_scalar(
                                hT8[:, c, :], psv, 1.0 / S, 0.0,
                                op0=OP.mult, op1=OP.max)
                    x2s = []
                    for j in range(NB):
                        ps = pspool.tile([P, H], FP32, tag="ps",
                                         name=f"ps_f2{st_}_{t}_{j}")
                        for cc in range(KF // 2):
                            nc.tensor.matmul(
                                ps[:], hT8[:, 2 * cc:2 * cc + 2, j * P:(j + 1) * P],
                                w2T8v[:, cc, :, :],
                                start=(cc == 0), stop=(cc == KF // 2 - 1),
                                perf_mode=DRPM)
                        x2 = rpool.tile([P, H], BF16, tag="x2", bufs=4,
                                        name=f"x2_{st_}_{t}_{j}")
                        nc.vector.scalar_tensor_tensor(
                            out=x2[:], in0=ps[:], scalar=1.0 / S,
                            in1=x_nat[:, t, j, :], op0=OP.mult, op1=OP.add)
                        x2s.append(x2)
                    rstd, nmr = ln_stats(x2s, f"l2_{st_}_{t}")
                    for j in range(NB):
                        if t == 0:
                            nc.scalar.activation(pooled[:, j, :], x2s[j][:],
                                                 AF.Identity, bias=nmr[:, j:j + 1],
                                                 scale=rstd[:, j:j + 1])
                        else:
                            n2t = rpool.tile([P, H], BF16, tag="n2t", bufs=1,
                                             name=f"n2_{st_}_{t}_{j}")
                            nc.scalar.activation(n2t[:], x2s[j][:],
                                                 AF.Identity, bias=nmr[:, j:j + 1],
                                                 scale=rstd[:, j:j + 1])
                            nc.vector.tensor_add(pooled[:, j, :],
                                                 pooled[:, j, :], n2t[:])

                # ---- stage 7: add pooled/3 into the early final partial ----
                for j in range(NB):
                    nc.vector.scalar_tensor_tensor(
                        out=outt[:, j, :], in0=pooled[:, j, :],
                        scalar=1.0 / 3, in1=outt[:, j, :],
                        op0=OP.mult, op1=OP.add)
                nc.sync.dma_start(
                    out_d[r0:r0 + ST].rearrange("(j p) f -> p j f", p=P),
                    outt[:])
                if st_ + 1 < n_st:
                    featsT = featsT_next

            for mp in reversed(main_pools):
                mp.__exit__(None, None, None)

    nc.compile()
    return nc


_CACHE = {}


def _get_nc(b_core, ln1_affine, ln2_affine, fast):
    key = (b_core, ln1_affine, ln2_affine, fast)
    if key not in _CACHE:
        if fast:
            _CACHE[key] = build_fast(b_core)
        else:
            _CACHE[key] = build_nc(b_core, ln1_affine, ln2_affine)
    return _CACHE[key]


def kernel(**inputs):
    inp = {k: np.asarray(v, dtype=np.float32) for k, v in inputs.items()}
    ln1_affine = not (np.all(inp["g1"] == 1.0) and np.all(inp["be1"] == 0.0))
    ln2_affine = not np.all(inp["g2"] == 1.0)
    zero_bias = all(
        np.all(inp[nm] == 0.0)
        for nm in ("bv", "bt", "ba", "in_proj_b", "out_b", "b1", "b2",
                   "be2", "bo"))
    fast = zero_bias and not ln1_affine and not ln2_affine

    nc = _get_nc(B_CORE, ln1_affine, ln2_affine, fast)

    shared = {k: inp[k] for k in inp
              if k not in ("visual_feat", "tactile_feat", "arm_feat")}
    in_maps = []
    for c in range(N_CORES):
        sl = slice(c * B_CORE, (c + 1) * B_CORE)
        m = dict(shared)
        m["visual_feat"] = np.ascontiguousarray(inp["visual_feat"][sl])
        m["tactile_feat"] = np.ascontiguousarray(inp["tactile_feat"][sl])
        m["arm_feat"] = np.ascontiguousarray(inp["arm_feat"][sl])
        in_maps.append(m)

    res = run_bass_kernel_spmd(nc, in_maps, core_ids=list(range(N_CORES)))
    return np.concatenate([res.results[c]["out"] for c in range(N_CORES)], axis=0)

